# revision 77
# baseline (speedup 1.0000x reference)
"""Self-contained Trainium2 (Bass/Tile) kernel for nn_CausalSTDiTBlock_80058190397994.

kernel(**inputs) takes the FULL unsharded inputs (x, y, t, tpe, sst, weights)
and returns the full (4, 4096, 1152) float32 output, running SPMD across 8
NeuronCores. Sharding: core = (batch, spatial-half); AdaLN modulation /
gates are folded into per-core host-prepped weights and biases.

Device design notes:
  Residual x_res is feature-major (C x 2048) bf16, SBUF-resident.
  K biases are dropped everywhere (softmax-invariant: they add a per-query
  constant to all scores). V biases fold into the following projection bias
  (softmax weights sum to one). tpe is added to the input of the temporal
  branch during virtual-order staging, so temporal Q/K/V use plain weights.
  qT_*/kT_* feature-major (C x ntok) bf16 in DRAM; v_* token-major
  (ntok x C) bf16. Temporal tensors use virtual order v = g*128 + t*8+sig
  (g = group of 8 spatial locations); causal+block mask applied per tile.
  Attention runs in waves of 4 heads: scores into one PSUM bank, one Exp
  over [128, 4*128], PV + denominator matmuls, batched normalize.
  Weights for feature-major projections are host-relaid so each m-tile load
  is one contiguous descriptor per partition.
"""
import sys
sys.path.insert(0, "/opt/trn_rl_repo")
import numpy as np
from contextlib import ExitStack

import concourse.bass as bass
import concourse.mybir as mybir
import concourse.tile as tile
from concourse import bacc
from concourse.masks import make_identity

P = 128
T, C, NH, HD, YL = 16, 1152, 16, 72, 120
S, SH = 256, 128
NTOK = T * SH
GRP, NGRP = 8, 16
NC_C = C // P            # 9
NTT = NTOK // P          # 16
bf16 = mybir.dt.bfloat16
f32 = mybir.dt.float32
AF = mybir.ActivationFunctionType
ALU = mybir.AluOpType
SCALE = float(HD) ** -0.5
NW = 4                   # heads per attention wave


def build(replicate: int = 1):
    nc = bacc.Bacc(num_devices=8)
    dp = lambda name, shape, dt: nc.declare_dram_parameter(name, list(shape), dt, isOutput=False)

    xT_own = dp("xT_own", (C, NTOK), bf16)
    xT_prt = dp("xT_prt", (C, NTOK), bf16)
    yT = dp("yT", (C, YL), bf16)
    # m-major relaid weights for feature-major projections:
    #   w[p, m, k, j] = W_T[k*128+p, m*128+j]  (one contiguous desc/partition)
    wq_s = dp("wq_s", (P, NC_C * NC_C * P), bf16)
    wk_s = dp("wk_s", (P, NC_C * NC_C * P), bf16)
    wproj_s = dp("wproj_s", (P, NC_C * NC_C * P), bf16)
    wqk_t = dp("wqk_t", (P, 2 * NC_C * NC_C * P), bf16)
    wproj_t = dp("wproj_t", (P, NC_C * NC_C * P), bf16)
    wq_c = dp("wq_c", (P, NC_C * NC_C * P), bf16)
    wk_c = dp("wk_c", (P, NC_C * NC_C * P), bf16)
    wproj_c = dp("wproj_c", (P, NC_C * NC_C * P), bf16)
    wfc1 = dp("wfc1", (P, 4 * NC_C * NC_C * P), bf16)
    wfc2 = dp("wfc2", (P, NC_C * 4 * NC_C * P), bf16)
    # j-contiguous layouts for token-major (V-style) projections:
    #   w[p, k, j] = W_T[k*128+p, j]
    wv_s = dp("wv_s", (P, NC_C * C), bf16)
    wv_t = dp("wv_t", (P, NC_C * C), bf16)
    wv_c = dp("wv_c", (P, NC_C * C), bf16)
    bq_s = dp("bq_s", (C,), f32)
    bqk_t = dp("bqk_t", (2 * C,), f32)
    bq_c = dp("bq_c", (C,), f32)
    bproj_s = dp("bproj_s", (C,), f32)
    bproj_t = dp("bproj_t", (C,), f32)
    bproj_c = dp("bproj_c", (C,), f32)
    bfc1 = dp("bfc1", (4 * C,), f32)
    bfc2 = dp("bfc2", (C,), f32)
    tpev = dp("tpev", (P, NC_C * T), bf16)   # tpe feature-major [p, k, t]
    maskneg = dp("maskneg", (P, P), bf16)
    outT = nc.declare_dram_parameter("outT", [C, NTOK], f32, isOutput=True)

    with ExitStack() as ctx:
        tc = ctx.enter_context(tile.TileContext(nc))
        dr = ctx.enter_context(tc.tile_pool(name="dr", bufs=1, space="DRAM"))
        qT_s = dr.tile([C, NTOK], bf16, tag="qT_s")
        kT_all = dr.tile([C, T * S], bf16, tag="kT_all")
        v_s = dr.tile([T * S, C], bf16, tag="v_s")
        qT_t = dr.tile([C, NTOK], bf16, tag="qT_t")
        kT_t = dr.tile([C, NTOK], bf16, tag="kT_t")
        v_t = dr.tile([NTOK, C], bf16, tag="v_t")
        qT_c = dr.tile([C, NTOK], bf16, tag="qT_c")
        kT_y = dr.tile([C, YL], bf16, tag="kT_y")
        v_y = dr.tile([YL, C], bf16, tag="v_y")
        hT = dr.tile([4 * C, NTOK], bf16, tag="hT")
        big = ctx.enter_context(tc.tile_pool(name="big", bufs=1))
        cons = ctx.enter_context(tc.tile_pool(name="cons", bufs=1))
        wp = ctx.enter_context(tc.tile_pool(name="wp", bufs=2))
        rp = ctx.enter_context(tc.tile_pool(name="rp", bufs=6))
        lnp = ctx.enter_context(tc.tile_pool(name="lnp", bufs=3))
        sp = ctx.enter_context(tc.tile_pool(name="sp", bufs=2))
        sqp = ctx.enter_context(tc.tile_pool(name="sqp", bufs=4))
        bcp = ctx.enter_context(tc.tile_pool(name="bcp", bufs=4))
        lrow = ctx.enter_context(tc.tile_pool(name="lrow", bufs=2))
        ap_ = ctx.enter_context(tc.tile_pool(name="ap", bufs=2))
        fr = ctx.enter_context(tc.tile_pool(name="fr", bufs=2))
        pp = ctx.enter_context(tc.tile_pool(name="pp", bufs=2, space="PSUM"))
        psc = ctx.enter_context(tc.tile_pool(name="psc", bufs=4, space="PSUM"))
        pov = ctx.enter_context(tc.tile_pool(name="pov", bufs=2, space="PSUM"))

        # ---------- constants ----------
        ident = cons.tile([P, P], bf16, tag="ident")
        make_identity(nc, ident[:])
        ones16 = cons.tile([P, 16], bf16, tag="ones16")
        nc.vector.memset(ones16[:], 1.0)
        mask4 = cons.tile([P, NW, P], bf16, tag="mask4")
        for i in range(NW):
            nc.sync.dma_start(out=mask4[:, i, :], in_=maskneg[:, :])
        tpev_sb = cons.tile([P, NC_C, T], bf16, tag="tpev")
        nc.sync.dma_start(out=tpev_sb[:], in_=tpev.rearrange("p (k t) -> p k t", t=T))
        eps_t = cons.tile([1, 1], f32, tag="eps")
        nc.vector.memset(eps_t[:], 1e-6)

        def bias_cols(src, n, tag):
            t_ = cons.tile([P, n // P], f32, tag=tag)
            nc.gpsimd.dma_start(out=t_[:], in_=src.rearrange("(j p) -> p j", p=P))
            return t_
        b_q_s = bias_cols(bq_s, C, "b_q_s")
        b_qk_t = bias_cols(bqk_t, 2 * C, "b_qk_t")
        b_q_c = bias_cols(bq_c, C, "b_q_c")
        b_proj_s = bias_cols(bproj_s, C, "b_proj_s")
        b_proj_t = bias_cols(bproj_t, C, "b_proj_t")
        b_proj_c = bias_cols(bproj_c, C, "b_proj_c")
        b_fc1 = bias_cols(bfc1, 4 * C, "b_fc1")
        b_fc2 = bias_cols(bfc2, C, "b_fc2")

        x_res = big.tile([P, NC_C, NTOK], bf16, tag="xres")
        k_all_r = kT_all.rearrange("c (t s) -> c t s", t=T)

        def layer_norm(src_get, dst):
            """src_get(i, ch) -> bf16 AP (128 x 512); dst big [P,NC_C,NTOK] bf16.
            Two passes: all chunks' stats+broadcasts issue before any apply,
            so PE's stats matmuls never queue behind DVE apply chains."""
            bcs = []
            for ch in range(NTOK // 512):
                ps1 = pp.tile([1, 512], f32, tag="ps")
                ps2 = pp.tile([1, 512], f32, tag="ps")
                for i in range(NC_C):
                    xs = src_get(i, ch)
                    sq = sqp.tile([P, 512], bf16, tag="ln_sq")
                    if i % 2:
                        nc.scalar.activation(sq[:], xs, AF.Square)
                    else:
                        nc.vector.tensor_mul(sq[:], xs, xs)
                    nc.tensor.matmul(ps1[:], ones16[:, 0:1], xs,
                                     start=(i == 0), stop=(i == NC_C - 1))
                    nc.tensor.matmul(ps2[:], ones16[:, 0:1], sq[:],
                                     start=(i == 0), stop=(i == NC_C - 1))
                ra = lrow.tile([1, 512], f32, tag="ln_a")
                rb = lrow.tile([1, 512], f32, tag="ln_b")
                rc = lrow.tile([1, 512], f32, tag="ln_c")
                rd = lrow.tile([1, 512], bf16, tag="ln_d")
                nc.vector.tensor_scalar_mul(out=ra[:], in0=ps1[:], scalar1=1.0 / C)  # mu
                nc.vector.tensor_mul(rc[:], ra[:], ra[:])                            # mu^2
                nc.vector.scalar_tensor_tensor(out=rb[:], in0=ps2[:], scalar=1.0 / C,
                                               in1=rc[:], op0=ALU.mult,
                                               op1=ALU.subtract)                     # var
                nc.scalar.activation(rb[:], rb[:], AF.Sqrt, bias=eps_t[:])           # sd
                nc.vector.reciprocal(rc[:], rb[:])                                   # r
                nc.vector.tensor_mul(rd[:], ra[:], rc[:])                            # mu*r (bf16)
                rce = lrow.tile([1, 512], bf16, tag="ln_e")
                nc.vector.tensor_copy(out=rce[:], in_=rc[:])
                rbc = bcp.tile([P, 512], bf16, tag="ln_rbc")
                nc.gpsimd.partition_broadcast(rbc[:], rce[:])
                mbc = bcp.tile([P, 512], bf16, tag="ln_mbc")
                nc.gpsimd.partition_broadcast(mbc[:], rd[:])
                bcs.append((rbc, mbc))
            for ch, (rbc, mbc) in enumerate(bcs):
                for i in range(NC_C):
                    d = dst[:, i, ch * 512:(ch + 1) * 512]
                    eng = nc.gpsimd if i % 3 == 2 else nc.vector
                    eng.tensor_mul(d, src_get(i, ch), rbc[:])
                    eng.tensor_sub(d, d, mbc[:])

        def src_own(i, ch):
            return x_res[:, i, ch * 512:(ch + 1) * 512]

        def src_prt(i, ch):
            t_ = lnp.tile([P, 512], bf16, tag="ln_src")
            nc.sync.dma_start(out=t_[:], in_=xT_prt[i * P:(i + 1) * P,
                                                    ch * 512:(ch + 1) * 512])
            return t_[:]

        def w_fm(w_dram, m, ktiles=NC_C):
            """Load m-major relaid weight tile [P, ktiles, 128]."""
            wt = wp.tile([P, ktiles, P], bf16, tag="w", name="wt")
            nc.sync.dma_start(
                out=wt[:],
                in_=w_dram[:, m * ktiles * P:(m + 1) * ktiles * P]
                    .rearrange("p (k j) -> p k j", j=P))
            return wt

        def proj_fm(w_dram, rhs_get, m_tiles, evict_fn, n_tok=NTOK):
            nch = (n_tok + 511) // 512
            for m in range(m_tiles):
                wt = w_fm(w_dram, m)
                for ch in range(nch):
                    cw = min(512, n_tok - ch * 512)
                    ps = pp.tile([P, 512], f32, tag="ps")
                    for k in range(NC_C):
                        nc.tensor.matmul(ps[:, :cw], wt[:, k, :],
                                         rhs_get(k, ch, cw),
                                         start=(k == 0), stop=(k == NC_C - 1))
                    evict_fn(m, ch, ps, cw)

        def ev_plain(dram, bias_t, m, ch, ps, cw, evq=None):
            st = sp.tile([P, 512], bf16, tag="st")
            if bias_t is None:
                nc.scalar.activation(st[:, :cw], ps[:, :cw], AF.Copy)
            else:
                nc.scalar.activation(st[:, :cw], ps[:, :cw], AF.Identity,
                                     bias=bias_t[:, m:m + 1])
            (evq or nc.gpsimd).dma_start(
                out=dram[m * P:(m + 1) * P, ch * 512:ch * 512 + cw],
                in_=st[:, :cw])

        def proj_v(w_dram, lhs_of, m_tiles, dst_row_of, mrows=P):
            """lhs_of(m) -> list over k of (128 x mrows) lhsT APs;
            dst_row_of(m) -> (dram_tensor, row0)."""
            for nch in range(3):
                c0, cw = nch * 512, min(512, C - nch * 512)
                wt = wp.tile([P, NC_C, 512], bf16, tag="wbig", bufs=2)
                nc.sync.dma_start(
                    out=wt[:, :, :cw],
                    in_=w_dram.rearrange("p (k j) -> p k j", j=C)[:, :, c0:c0 + cw])
                for m in range(m_tiles):
                    ps = pp.tile([P, 512], f32, tag="ps")
                    lhs = lhs_of(m)
                    for k in range(NC_C):
                        nc.tensor.matmul(ps[:mrows, :cw], lhs[k], wt[:, k, :cw],
                                         start=(k == 0), stop=(k == NC_C - 1))
                    st = sp.tile([P, 512], bf16, tag="st")
                    nc.scalar.activation(st[:mrows, :cw], ps[:mrows, :cw], AF.Copy)
                    dram, row0 = dst_row_of(m)
                    nc.gpsimd.dma_start(
                        out=dram[row0:row0 + mrows, c0:c0 + cw],
                        in_=st[:mrows, :cw])

        for rep in range(replicate):
            for i in range(NC_C):
                eng = (nc.sync, nc.gpsimd, nc.scalar)[i % 3]
                eng.dma_start(out=x_res[:, i, :], in_=xT_own[i * P:(i + 1) * P, :])

            # =================== LayerNorm + projection emission ===============
            x_ln = big.tile([P, NC_C, NTOK], bf16, tag="xact")
            layer_norm(src_own, x_ln)
            x_ln_prt = big.tile([P, NC_C, NTOK], bf16, tag="prt")

            # cross-attention K/V depend only on y — project them early (their
            # PE work fills the first LayerNorm's stats stalls) and keep the
            # results SBUF-resident for the cross phase.
            y_sb = fr.tile([P, NC_C, YL], bf16, tag="y_sb", bufs=1)
            for k in range(NC_C):
                nc.sync.dma_start(out=y_sb[:, k, :], in_=yT[k * P:(k + 1) * P, :])
            rhs_y = lambda k, ch, cw: y_sb[:, k, :]
            proj_fm(wk_c, rhs_y, NC_C,
                    lambda m, ch, ps, cw: ev_plain(kT_y, None, m, ch, ps, cw),
                    n_tok=YL)

            def lhs_y(m):
                return [y_sb[:, k, :] for k in range(NC_C)]
            proj_v(wv_c, lhs_y, 1, lambda m: (v_y, 0), mrows=YL)

            ky_sb = cons.tile([HD, NH, YL], bf16, tag="ky_sb")
            nc.sync.dma_start(out=ky_sb[:],
                              in_=kT_y.rearrange("(h j) n -> j h n", j=HD))
            vy_sb = cons.tile([P, C], bf16, tag="vy_sb")
            nc.sync.dma_start(out=vy_sb[:YL, :], in_=v_y[:, :])

            # ---- spatial K own/prt -> kT_all cols [own|prt] per frame ----
            rhs_xln = lambda k, ch, cw: x_ln[:, k, ch * 512:ch * 512 + cw]
            rhs_prt = lambda k, ch, cw: x_ln_prt[:, k, ch * 512:ch * 512 + cw]

            def ev_k(side):
                def ev(m, ch, ps, cw):
                    st = sp.tile([P, 512], bf16, tag="st")
                    nc.scalar.activation(st[:, :cw], ps[:, :cw], AF.Copy)
                    nc.gpsimd.dma_start(
                        out=k_all_r[m * P:(m + 1) * P, ch * 4:(ch + 1) * 4,
                                    side * SH:(side + 1) * SH],
                        in_=st[:].rearrange("p (t s) -> p t s", s=SH))
                return ev
            proj_fm(wk_s, rhs_xln, NC_C, ev_k(0))

            def lhs_xln(m):
                return [x_ln[:, k, m * P:(m + 1) * P] for k in range(NC_C)]

            def lhs_prt(m):
                return [x_ln_prt[:, k, m * P:(m + 1) * P] for k in range(NC_C)]

            proj_v(wv_s, lhs_xln, NTT, lambda m: (v_s, m * S))
            proj_fm(wq_s, rhs_xln, NC_C,
                    lambda m, ch, ps, cw: ev_plain(qT_s, b_q_s, m, ch, ps, cw))
            # partner-half LN issued AFTER own projections: its DVE work
            # overlaps the own-token matmul phase, so PE never heads-of-line
            # blocks on the LN chain.
            layer_norm(src_prt, x_ln_prt)
            proj_fm(wk_s, rhs_prt, NC_C, ev_k(1))
            proj_v(wv_s, lhs_prt, NTT, lambda m: (v_s, m * S + SH))

            # =================== attention (wave-batched) ===================
            def attention(oT_dst, nk, q_of, k_of, v_of, masked, n_qb=NTT,
                          k_res=None, v_res=None, k_of_b=None, v_of_b=None):
                """Per (qb, wave of NW heads): q_of(qb,w)->AP [HD,NW,P];
                k_of(qb,w)->AP [HD,NW,nk]; v_of(qb,w)->AP [rows, NW*HD]
                (all DMA'd); or resident SBUF k_res [HD,NH,nk] /
                v_res [rows, C]. masked: apply mask4 (+SCALE) before exp."""
                nkt = (nk + P - 1) // P
                for qb in range(n_qb):
                    o_acc = ap_.tile([P, C], bf16, tag="o_acc")
                    for w in range(NH // NW):
                        qt = fr.tile([HD, NW, P], bf16, tag="qt", name="qt")
                        (nc.sync if (qb + w) % 2 else nc.gpsimd).dma_start(
                            out=qt[:], in_=q_of(qb, w))
                        if k_res is None:
                            kt = fr.tile([HD, NW, nkt * P], bf16, tag="kt",
                                         name="kt")
                            if k_of_b is None:
                                (nc.gpsimd if (qb + w) % 2 else nc.sync).dma_start(
                                    out=kt[:, :, :nk], in_=k_of(qb, w))
                            else:
                                nc.sync.dma_start(out=kt[:, :, :P],
                                                  in_=k_of(qb, w))
                                nc.gpsimd.dma_start(out=kt[:, :, P:2 * P],
                                                    in_=k_of_b(qb, w))
                            kt_ap = kt
                        else:
                            kt_ap = k_res[:, w * NW:(w + 1) * NW, :]
                        if v_res is None:
                            vt = fr.tile([P, nkt, NW * HD], bf16, tag="vt",
                                         name="vt")
                            if v_of_b is None:
                                (nc.sync if w % 2 else nc.gpsimd).dma_start(
                                    out=vt[:] if nkt > 1 else vt[:nk, 0, :],
                                    in_=v_of(qb, w))
                            else:
                                nc.gpsimd.dma_start(out=vt[:, 0, :],
                                                    in_=v_of(qb, w))
                                nc.sync.dma_start(out=vt[:, 1, :],
                                                  in_=v_of_b(qb, w))
                            vt_of = lambda kt_i: vt[:, kt_i, :]
                        else:
                            vt_of = lambda kt_i: v_res[:, w * NW * HD:
                                                       (w + 1) * NW * HD]
                        # one accumulation group per sc tile: start=True zeroes
                        # the whole 2KB zero region, so only the first matmul
                        # into each tile may carry start (heads write disjoint
                        # column ranges of the zeroed bank).
                        sc = [psc.tile([P, NW * P], f32, tag="sc", name="sc")
                              for _ in range(nkt)]
                        for i in range(NW):
                            for kt_i in range(nkt):
                                kp = min(P, nk - kt_i * P)
                                nc.tensor.matmul(
                                    sc[kt_i][:kp, i * P:(i + 1) * P],
                                    kt_ap[:, i, kt_i * P:kt_i * P + kp],
                                    qt[:, i, :], start=(i == 0),
                                    stop=(i == NW - 1))
                        es = []
                        for kt_i in range(nkt):
                            kp = min(P, nk - kt_i * P)
                            e = ap_.tile([P, NW * P], bf16, tag="e%d" % kt_i,
                                         name="e")
                            if masked:
                                nc.vector.scalar_tensor_tensor(
                                    out=sc[kt_i][:kp, :], in0=sc[kt_i][:kp, :],
                                    scalar=SCALE, in1=mask4[:kp, :, :]
                                    .rearrange("p w j -> p (w j)"),
                                    op0=ALU.mult, op1=ALU.add)
                                nc.scalar.activation(e[:kp, :], sc[kt_i][:kp, :],
                                                     AF.Exp)
                            else:
                                nc.scalar.activation(e[:kp, :], sc[kt_i][:kp, :],
                                                     AF.Exp, scale=SCALE)
                            es.append(e)
                        # single accumulation group over the whole ov bank:
                        # first matmul zeroes it, every later one accumulates
                        # into its disjoint range, last one closes the group.
                        ov = pov.tile([P, 512], f32, tag="ov")
                        for i in range(NW):
                            for kt_i in range(nkt):
                                kp = min(P, nk - kt_i * P)
                                nc.tensor.matmul(
                                    ov[:, i * HD:(i + 1) * HD],
                                    es[kt_i][:kp, i * P:(i + 1) * P],
                                    vt_of(kt_i)[:kp, i * HD:(i + 1) * HD],
                                    start=(i == 0 and kt_i == 0), stop=False)
                                nc.tensor.matmul(
                                    ov[:, NW * HD + i:NW * HD + i + 1],
                                    es[kt_i][:kp, i * P:(i + 1) * P],
                                    ones16[:kp, 0:1],
                                    start=False,
                                    stop=(i == NW - 1 and kt_i == nkt - 1))
                        rec = ap_.tile([P, NW], f32, tag="rec")
                        nc.vector.reciprocal(rec[:], ov[:, NW * HD:NW * HD + NW])
                        for i in range(NW):
                            h = w * NW + i
                            nc.vector.tensor_scalar_mul(
                                out=o_acc[:, h * HD:(h + 1) * HD],
                                in0=ov[:, i * HD:(i + 1) * HD],
                                scalar1=rec[:, i:i + 1])
                    for cb in range(NC_C):
                        tp = pov.tile([P, P], bf16, tag="ov", name="tp")
                        nc.tensor.transpose(tp[:], o_acc[:, cb * P:(cb + 1) * P],
                                            ident[:])
                        if cb % 2:
                            nc.scalar.copy(oT_dst[:, cb, qb * P:(qb + 1) * P], tp[:])
                        else:
                            nc.vector.tensor_copy(
                                out=oT_dst[:, cb, qb * P:(qb + 1) * P], in_=tp[:])

            # ---- spatial attention ----
            q_s_r = qT_s.rearrange("(h j) (t s) -> j h t s", j=HD, t=T)
            k_sr = kT_all.rearrange("(h j) (t s) -> j h t s", j=HD, t=T)
            v_sr = v_s.rearrange("(t k p) c -> t p k c", k=2, p=P)
            oT_sp = big.tile([P, NC_C, NTOK], bf16, tag="prt")
            attention(
                oT_sp, S,
                q_of=lambda qb, w: q_s_r[:, w * NW:(w + 1) * NW, qb, :],
                k_of=lambda qb, w: k_sr[:, w * NW:(w + 1) * NW, qb, :],
                v_of=lambda qb, w: v_sr[qb][:, :, w * NW * HD:(w + 1) * NW * HD],
                masked=False)

            # ---- residual projection (feature-major into x_res) ----
            def proj_residual(w_dram, rhs_big, bias_t, scatter=False):
                for m in range(NC_C):
                    wt = w_fm(w_dram, m)
                    for ch in range(NTOK // 512):
                        ps = pp.tile([P, 512], f32, tag="ps")
                        for k in range(NC_C):
                            nc.tensor.matmul(ps[:], wt[:, k, :],
                                             rhs_big[:, k, ch * 512:(ch + 1) * 512],
                                             start=(k == 0), stop=(k == NC_C - 1))
                        if not scatter:
                            nc.vector.scalar_tensor_tensor(
                                out=x_res[:, m, ch * 512:(ch + 1) * 512],
                                in0=ps[:], scalar=bias_t[:, m:m + 1],
                                in1=x_res[:, m, ch * 512:(ch + 1) * 512],
                                op0=ALU.add, op1=ALU.add)
                        else:
                            xr = x_res[:, m, :].rearrange("p (t s) -> p t s", t=T)
                            for g4 in range(4):
                                g = ch * 4 + g4
                                nc.vector.scalar_tensor_tensor(
                                    out=xr[:, :, g * GRP:(g + 1) * GRP],
                                    in0=ps[:, g4 * P:(g4 + 1) * P]
                                        .rearrange("p (t s) -> p t s", s=GRP),
                                    scalar=bias_t[:, m:m + 1],
                                    in1=xr[:, :, g * GRP:(g + 1) * GRP],
                                    op0=ALU.add, op1=ALU.add)

            proj_residual(wproj_s, oT_sp, b_proj_s)

            # =================== temporal ===================
            # stage x_res + tpe into temporal-virtual token order
            x_virt = big.tile([P, NC_C, NTOK], bf16, tag="prt")
            for k in range(NC_C):
                xrk = x_res[:, k, :].rearrange("p (t s) -> p t s", t=T)
                xvk = x_virt[:, k, :].rearrange("p (g t s) -> p g t s", g=NGRP, t=T)
                tk = tpev_sb[:, k, :].rearrange("p (t o) -> p t o", o=1)
                for g in range(NGRP):
                    eng = nc.vector if g % 2 else nc.gpsimd
                    eng.tensor_add(
                        out=xvk[:, g, :, :],
                        in0=xrk[:, :, g * GRP:(g + 1) * GRP],
                        in1=tk.broadcast_to([P, T, GRP]))

            rhs_virt = lambda k, ch, cw: x_virt[:, k, ch * 512:ch * 512 + cw]

            def lhs_virt(g):
                return [x_virt[:, k, g * P:(g + 1) * P] for k in range(NC_C)]

            def w_qk_t(m):
                return w_fm(wqk_t, m)

            proj_fm(wqk_t, rhs_virt, NC_C,
                    lambda m, ch, ps, cw: ev_plain(qT_t, b_qk_t, m, ch, ps, cw))

            def ev_kt(m, ch, ps, cw):
                st = sp.tile([P, 512], bf16, tag="st")
                nc.scalar.activation(st[:, :cw], ps[:, :cw], AF.Copy)
                nc.gpsimd.dma_start(out=kT_t[m * P:(m + 1) * P, ch * 512:ch * 512 + cw],
                                    in_=st[:, :cw])

            # K tiles live in the second half of wqk_t (m offset NC_C)
            for m in range(NC_C):
                wt = w_fm(wqk_t, NC_C + m)
                for ch in range(NTOK // 512):
                    ps = pp.tile([P, 512], f32, tag="ps")
                    for k in range(NC_C):
                        nc.tensor.matmul(ps[:], wt[:, k, :],
                                         rhs_virt(k, ch, 512),
                                         start=(k == 0), stop=(k == NC_C - 1))
                    ev_kt(m, ch, ps, 512)

            proj_v(wv_t, lhs_virt, NGRP, lambda g: (v_t, g * P))

            q_t_r = qT_t.rearrange("(h j) n -> j h n", j=HD)
            k_t_r = kT_t.rearrange("(h j) n -> j h n", j=HD)
            oT_t = big.tile([P, NC_C, NTOK], bf16, tag="xact")
            attention(
                oT_t, P,
                q_of=lambda qb, w: q_t_r[:, w * NW:(w + 1) * NW, qb * P:(qb + 1) * P],
                k_of=lambda qb, w: k_t_r[:, w * NW:(w + 1) * NW, qb * P:(qb + 1) * P],
                v_of=lambda qb, w: v_t[qb * P:(qb + 1) * P,
                                       w * NW * HD:(w + 1) * NW * HD],
                masked=True, n_qb=NGRP)
            proj_residual(wproj_t, oT_t, b_proj_t, scatter=True)

            # =================== cross ===================
            rhs_xres = lambda k, ch, cw: x_res[:, k, ch * 512:ch * 512 + cw]
            proj_fm(wq_c, rhs_xres, NC_C,
                    lambda m, ch, ps, cw: ev_plain(qT_c, b_q_c, m, ch, ps, cw))

            q_c_r = qT_c.rearrange("(h j) n -> j h n", j=HD)
            oT_c = big.tile([P, NC_C, NTOK], bf16, tag="prt")
            attention(
                oT_c, YL,
                q_of=lambda qb, w: q_c_r[:, w * NW:(w + 1) * NW, qb * P:(qb + 1) * P],
                k_of=None, v_of=None,
                masked=False, k_res=ky_sb[:], v_res=vy_sb[:])
            proj_residual(wproj_c, oT_c, b_proj_c)

            # =================== MLP ===================
            x_ln2 = big.tile([P, NC_C, NTOK], bf16, tag="xact")
            layer_norm(src_own, x_ln2)
            rhs_xln2 = lambda k, ch, cw: x_ln2[:, k, ch * 512:ch * 512 + cw]

            def ev_gelu(m, ch, ps, cw):
                st = sp.tile([P, 512], bf16, tag="st")
                nc.scalar.activation(st[:, :cw], ps[:, :cw], AF.Gelu_apprx_tanh,
                                     bias=b_fc1[:, m:m + 1])
                nc.gpsimd.dma_start(out=hT[m * P:(m + 1) * P, ch * 512:ch * 512 + cw],
                                    in_=st[:, :cw])
            proj_fm(wfc1, rhs_xln2, 4 * C // P, ev_gelu)

            def load_w2(m):
                wt = wp.tile([P, 4 * NC_C, P], bf16, tag="wbig", bufs=2, name="wt")
                nc.scalar.dma_start(out=wt[:],
                                    in_=wfc2[:, m * 4 * NC_C * P:(m + 1) * 4 * NC_C * P]
                                    .rearrange("p (k j) -> p k j", j=P))
                return wt

            w2_next = load_w2(0)
            for m in range(NC_C):
                wt = w2_next
                for ch in range(NTOK // 512):
                    if ch == 1 and m + 1 < NC_C:
                        w2_next = load_w2(m + 1)
                    ps = pp.tile([P, 512], f32, tag="ps")
                    for k2 in range(4 * C // P // 2):
                        rh = rp.tile([P, 2, 512], bf16, tag="rh")
                        eng = (nc.sync, nc.gpsimd, nc.scalar)[k2 % 3]
                        eng.dma_start(out=rh[:],
                                      in_=hT[k2 * 2 * P:(k2 + 1) * 2 * P,
                                             ch * 512:(ch + 1) * 512]
                                          .rearrange("(o p) n -> p o n", p=P))
                        for kk in range(2):
                            k = k2 * 2 + kk
                            nc.tensor.matmul(ps[:], wt[:, k, :], rh[:, kk, :],
                                             start=(k == 0), stop=(k == 4 * C // P - 1))
                    so = sp.tile([P, 512], f32, tag="st_out")
                    nc.vector.scalar_tensor_tensor(
                        out=so[:],
                        in0=ps[:], scalar=b_fc2[:, m:m + 1],
                        in1=x_res[:, m, ch * 512:(ch + 1) * 512],
                        op0=ALU.add, op1=ALU.add)
                    (nc.sync if (m + ch) % 2 else nc.gpsimd).dma_start(
                        out=outT[m * P:(m + 1) * P, ch * 512:(ch + 1) * 512],
                        in_=so[:])

    nc.finalize()
    return nc


# ======================= SPMD runner =======================
import time
import jax
from jax.sharding import Mesh, PartitionSpec
from jax.experimental.shard_map import shard_map
from concourse.bass2jax import _bass_exec_p, install_neuronx_cc_hook, partition_id_tensor

def make_runner(nc: bass.Bass, n_cores: int = 8):
    install_neuronx_cc_hook()
    assert nc.dbg_addr is None or not nc.dbg_callbacks

    partition_name = nc.partition_id_tensor.name if nc.partition_id_tensor else None
    in_names, out_names, out_avals, zero_outs = [], [], [], []
    for alloc in nc.m.functions[0].allocations:
        if not isinstance(alloc, mybir.MemoryLocationSet):
            continue
        name = alloc.memorylocations[0].name
        if alloc.kind == "ExternalInput":
            if name != partition_name:
                in_names.append(name)
        elif alloc.kind == "ExternalOutput":
            out_names.append(name)
            shape = tuple(alloc.tensor_shape)
            dtype = mybir.dt.np(alloc.dtype)
            out_avals.append(jax.core.ShapedArray(shape, dtype))
            zero_outs.append(np.zeros(shape, dtype))
    n_params = len(in_names)
    n_outs = len(out_avals)
    all_in_names = list(in_names) + list(out_names)
    if partition_name is not None:
        all_in_names.append(partition_name)

    def _body(*args):
        operands = list(args)
        if partition_name is not None:
            operands.append(partition_id_tensor())
        outs = _bass_exec_p.bind(
            *operands,
            out_avals=tuple(out_avals),
            in_names=tuple(all_in_names),
            out_names=tuple(out_names),
            lowering_input_output_aliases=(),
            sim_require_finite=True,
            sim_require_nnan=True,
            nc=nc,
        )
        return tuple(outs)

    devices = jax.devices()[:n_cores]
    mesh = Mesh(np.asarray(devices), ("core",))
    in_specs = (PartitionSpec("core"),) * (n_params + n_outs)
    out_specs = (PartitionSpec("core"),) * n_outs
    donate = tuple(range(n_params, n_params + n_outs))
    sharded = jax.jit(
        shard_map(_body, mesh=mesh, in_specs=in_specs, out_specs=out_specs,
                  check_rep=False),
        donate_argnums=donate, keep_unused=True,
    )

    sharding = jax.sharding.NamedSharding(mesh, PartitionSpec("core"))

    def run(in_maps, n_iters=3):
        per_core = [[np.asarray(m[name]) for name in in_names] for m in in_maps]
        concat_in = [
            np.concatenate([per_core[c][i] for c in range(n_cores)], axis=0)
            for i in range(n_params)
        ]
        dev_in = [jax.device_put(a, sharding) for a in concat_in]
        times = []
        out_arrs = None
        for it in range(n_iters):
            dev_zeros = [
                jax.device_put(np.zeros((n_cores * z.shape[0], *z.shape[1:]), z.dtype),
                               sharding)
                for z in zero_outs
            ]
            for z in dev_zeros:
                z.block_until_ready()
            t0 = time.perf_counter()
            out = sharded(*dev_in, *dev_zeros)
            for o in out:
                o.block_until_ready()
            t1 = time.perf_counter()
            times.append(t1 - t0)
            out_arrs = out
        results = [
            {
                name: np.asarray(out_arrs[i]).reshape(n_cores, *out_avals[i].shape)[c]
                for i, name in enumerate(out_names)
            }
            for c in range(n_cores)
        ]
        return results, times

    return run


# ======================= host prep + entry point =======================
import ml_dtypes

B = 4
bfloat16 = ml_dtypes.bfloat16


def _bf(x):
    return np.ascontiguousarray(x, dtype=np.float32).astype(bfloat16)


def _fm(wT):
    """Relay W_T (C x M) into m-major [p, m, k, j] device layout."""
    Cin, M = wT.shape
    nk, nm = Cin // P, M // P
    w = wT.reshape(nk, P, nm, P).transpose(1, 2, 0, 3).reshape(P, nm * nk * P)
    return _bf(w)


def _jc(wT):
    """Relay W_T (C x M) into j-contiguous [p, k, j] device layout."""
    Cin, M = wT.shape
    nk = Cin // P
    w = wT.reshape(nk, P, M).transpose(1, 0, 2).reshape(P, nk * M)
    return _bf(w)


def build_in_maps(inputs):
    x = np.asarray(inputs['x'], np.float32)
    y = np.asarray(inputs['y'], np.float32)
    t = np.asarray(inputs['t'], np.float32)
    tpe = np.asarray(inputs['tpe'], np.float32)
    sst = np.asarray(inputs['sst'], np.float32)
    W = {k: np.asarray(inputs[k], np.float32) for k in inputs
         if k not in ('x', 'y', 't', 'tpe', 'sst')}

    t6 = sst[None] + t.reshape(B, 6, C)
    sh_msa, sc_msa, g_msa, sh_mlp, sc_mlp, g_mlp = [t6[:, i] for i in range(6)]

    mask = np.zeros((P, P), np.float32)
    t2 = np.arange(P)[:, None] // GRP
    s2 = np.arange(P)[:, None] % GRP
    t1 = np.arange(P)[None, :] // GRP
    s1 = np.arange(P)[None, :] % GRP
    mask[~((s2 == s1) & (t2 <= t1))] = -30000.0

    # tpe feature-major [p, k, t]
    tpe_fm = tpe[0].T.reshape(NC_C, P, T).transpose(1, 0, 2).reshape(P, NC_C * T)

    in_maps = []
    for b in range(B):
        wqkv_s = W['qkv_s_w'] * (1.0 + sc_msa[b])[None, :]
        bqkv_s = W['qkv_s_w'] @ sh_msa[b] + W['qkv_s_b']
        wproj_s = W['proj_s_w'] * g_msa[b][:, None]
        bproj_s = g_msa[b] * W['proj_s_b'] + wproj_s @ bqkv_s[2 * C:]
        wproj_t = W['proj_t_w'] * g_msa[b][:, None]
        bproj_t = (g_msa[b] * W['proj_t_b']
                   + wproj_t @ W['qkv_t_b'][2 * C:])
        bv_c = W['kv_c_b'][C:]
        bproj_c = W['proj_c_b'] + W['proj_c_w'] @ bv_c
        wfc1 = W['fc1_w'] * (1.0 + sc_mlp[b])[None, :]
        bfc1 = W['fc1_w'] @ sh_mlp[b] + W['fc1_b']
        wfc2 = W['fc2_w'] * g_mlp[b][:, None]
        bfc2 = g_mlp[b] * W['fc2_b']

        common = dict(
            yT=_bf(y[b].T),
            wq_s=_fm(wqkv_s[:C].T), wk_s=_fm(wqkv_s[C:2 * C].T),
            wv_s=_jc(wqkv_s[2 * C:].T),
            wproj_s=_fm(wproj_s.T),
            wqk_t=_fm(W['qkv_t_w'][:2 * C].T), wv_t=_jc(W['qkv_t_w'][2 * C:].T),
            wproj_t=_fm(wproj_t.T),
            wq_c=_fm(W['q_c_w'].T), wk_c=_fm(W['kv_c_w'][:C].T),
            wv_c=_jc(W['kv_c_w'][C:].T), wproj_c=_fm(W['proj_c_w'].T),
            wfc1=_fm(wfc1.T), wfc2=_fm(wfc2.T),
            bq_s=np.ascontiguousarray(bqkv_s[:C], np.float32),
            bqk_t=np.ascontiguousarray(W['qkv_t_b'][:2 * C], np.float32),
            bq_c=np.ascontiguousarray(W['q_c_b'], np.float32),
            bproj_s=np.ascontiguousarray(bproj_s, np.float32),
            bproj_t=np.ascontiguousarray(bproj_t, np.float32),
            bproj_c=np.ascontiguousarray(bproj_c, np.float32),
            bfc1=np.ascontiguousarray(bfc1, np.float32),
            bfc2=np.ascontiguousarray(bfc2, np.float32),
            tpev=_bf(tpe_fm),
            maskneg=_bf(mask),
        )
        xb = x[b].reshape(T, S, C)
        for sh in range(2):
            own = xb[:, sh * SH:(sh + 1) * SH, :].reshape(NTOK, C)
            prt = xb[:, (1 - sh) * SH:(2 - sh) * SH, :].reshape(NTOK, C)
            m = dict(common)
            m['xT_own'] = _bf(own.T)
            m['xT_prt'] = _bf(prt.T)
            in_maps.append(m)
    return in_maps


def assemble(outs):
    xout = np.zeros((B, T * S, C), np.float32)
    ci = 0
    for b in range(B):
        for sh in range(2):
            o = outs[ci]['outT']            # (C, NTOK)
            tok = o.T.reshape(T, SH, C)
            xout[b].reshape(T, S, C)[:, sh * SH:(sh + 1) * SH, :] = tok
            ci += 1
    return xout


_CACHE = {}


def run_kernel(inputs, replicate=1, n_iters=2):
    key = replicate
    if key not in _CACHE:
        nc = build(replicate)
        _CACHE[key] = make_runner(nc, 8)
    run = _CACHE[key]
    in_maps = build_in_maps(inputs)
    results, times = run(in_maps, n_iters=n_iters)
    return assemble(results), times


def kernel(**inputs):
    out, _ = run_kernel(inputs, replicate=1, n_iters=1)
    return out
